# revision 1
# baseline (speedup 1.0000x reference)
"""GQA attention (B=2, S=2048, HID=2048, 32 q heads / 8 kv heads, fp32 I/O)
on 8 TRN2 NeuronCores.

Sharding: sequence-parallel with fully local K/V. Core c owns 512 query
tokens of batch c//4 (cores 0-3 = batch 0, cores 4-7 = batch 1), but
computes K^T and V for ALL 2048 tokens of its batch locally — that
(+~40% KV projection FLOPs) is much cheaper than an intra-chip
AllGather, which measures 100-170us and blockades the DMA engines while
it runs. Attention is permutation-invariant over keys, so each core
orders tokens own-block-first (host-side permutation) and the device
program stays rank-independent.

V carries a fused ones-column per kv head so the PV matmul also
produces the softmax row-sums; the output-projection bias is fused as
an extra contraction row. All matmuls run in bf16 with fp32 PSUM
accumulation (fp32 matmul is 4x slower on the PE).

Phase D is software-pipelined at (pair, 2-key-chunk) step granularity:
each step emits the PV matmuls for step s-2 FIRST (they never wait, so
they fill the PE's wait on exp freeing the score-PSUM ring), then
scores+exp for step s. Score PSUM tiles double-buffer at half-step
granularity (head A vs head B), keeping the ACT exp stream near
back-to-back. Background PE work — 14 Q-projection chunks and the K
mt2/mt3 projection groups — queues upfront in consumer-deadline order
and drips at 6 thunks/step, K units interleaved between Q chunks so
every accq PSUM-bank handoff gets a step of DMA-thunk separation
(phase A runs only K mt0/mt1 + all of V inline, starting exp ~30us
earlier). Softmax normalization for pair p is staged across pair
p+1's steps (PSUM->SBUF copy, reciprocal, cast, PE ones-broadcast,
multiply), so the 3.3us DVE reciprocal never blocks a PE-side
consumer.
"""

import functools
from collections import deque
from contextlib import ExitStack

import numpy as np
import ml_dtypes

import concourse.bass as bass
import concourse.mybir as mybir
import concourse.tile as tile
from concourse import bacc
from concourse.bass_utils import run_bass_kernel_spmd

BF = mybir.dt.bfloat16
F32 = mybir.dt.float32

B, S, HID = 2, 2048, 2048
NH, NKV, HD = 32, 8, 64          # q heads, kv heads, head dim
GRP = NH // NKV                  # 4 q heads per kv head
TP = 4                           # cores per batch group
TOK = S // TP                    # 512 local query tokens per core
KC = HID // 128                  # 16 contraction chunks of 128
NKC = S // 128                   # 16 key chunks of 128 (full seq)
VW = NKV * (HD + 1)              # 520: V width incl. ones columns
EXP_SCALE = float(HD) ** -0.5    # 1/8 softmax scale, fused into Exp


def q_slot(h):
    """qTp tile index and partition base for head h.

    Head h lives at partition base ((h//4)%2)*64 — the same base its kv
    head kh=h//4 occupies inside the kTg tiles, so the scores matmul's
    lhsT and rhs stay partition-aligned.
    """
    return ((h // 4) // 2) * 4 + (h % 4), ((h // 4) % 2) * 64


def build_graph():
    nc = bacc.Bacc(None, target_bir_lowering=False, debug=False, num_devices=8)

    xT = nc.declare_dram_parameter("xT", [HID, S], BF, isOutput=False)
    wkT = nc.declare_dram_parameter("wkT", [HID, NKV * HD], BF, isOutput=False)
    wvT = nc.declare_dram_parameter("wvT", [HID, NKV * HD], BF, isOutput=False)
    wqT = nc.declare_dram_parameter("wqT", [HID, HID], BF, isOutput=False)
    woT = nc.declare_dram_parameter("woT", [HID + 1, HID], BF, isOutput=False)
    out = nc.declare_dram_parameter("out", [TOK, HID], F32, isOutput=True)

    with tile.TileContext(nc) as tc, ExitStack() as es:
        pers = es.enter_context(tc.tile_pool(name="pers", bufs=1))

        def T(shape, dtype, *, name):
            return pers.tile(shape, dtype, name=name, tag=name)

        # ---- SBUF inputs; DMA issue order = priority --------------------
        # xq: own 512 query-token columns (live through attention for the
        # Q projection).  xr/wk/wv live only through phase A (pool xin
        # closes after it, freeing 80KB/partition for attention pools).
        xin = es.enter_context(tc.tile_pool(name="xin", bufs=1))
        xq = [T([128, TOK], BF, name=f"xq{k}") for k in range(KC)]
        xr = [xin.tile([128, S - TOK], BF, tag=f"xr{k}", name=f"xr{k}")
              for k in range(KC)]
        wk_sb = [xin.tile([128, NKV * HD], BF, tag=f"wk{k}", name=f"wk{k}")
                 for k in range(KC)]
        wv_sb = [xin.tile([128, NKV * HD], BF, tag=f"wv{k}", name=f"wv{k}")
                 for k in range(KC)]
        for k in range(KC):
            nc.sync.dma_start(out=wk_sb[k][:, :], in_=wkT[k * 128:(k + 1) * 128, :])
            nc.sync.dma_start(out=xq[k][:, :], in_=xT[k * 128:(k + 1) * 128, 0:TOK])
            nc.sync.dma_start(out=xr[k][:, :], in_=xT[k * 128:(k + 1) * 128, TOK:S])
        for k in range(KC):
            nc.sync.dma_start(out=wv_sb[k][:, :], in_=wvT[k * 128:(k + 1) * 128, :])

        def xcols(k, lo, n):
            # columns lo..lo+n of the permuted x^T chunk k
            return xq[k][:, lo:lo + n] if lo < TOK \
                else xr[k][:, lo - TOK:lo - TOK + n]

        # row HD (partition 64) is the K=1 lhsT for the row-sum broadcast
        ones64 = T([HD + 1, 64], BF, name="ones64")
        nc.vector.memset(ones64[:, :], 1.0)
        ones128 = T([1, 128], BF, name="ones128")
        nc.vector.memset(ones128[:, :], 1.0)

        # kTg[nb*4+mt]: [128, 512] = K^T rows mt*128.. for key block nb
        # (kv heads 2mt at partitions 0-63, 2mt+1 at 64-127).
        # vg[c]: [128, 520] V_aug rows for key chunk c, ones at col
        # kh*65+64 of each kv head kh.
        kTg = [T([128, TOK], BF, name=f"kTg{i}") for i in range(16)]
        vg = [T([128, VW], BF, name=f"vg{c}") for c in range(NKC)]
        qTp = [T([128, TOK], BF, name=f"qTp{i}") for i in range(NH // 2)]
        attnT = [T([128, TOK], BF, name=f"attnT{t}") for t in range(NH // 2)]

        # =============== phase A: K^T and V_aug for the whole batch ======
        # 4 waves of 8 PSUM accumulation groups, contraction-chunk OUTER:
        # the PE streams 8 matmuls per arriving x chunk instead of
        # stalling on the full 16-chunk DMA before the first group.
        # mt-major K so the kv-head tiles the first pair group needs land
        # first.
        with tc.tile_pool(name="accA", bufs=8, space="PSUM") as accA:
            # K mt2/mt3 (first needed by pair group 2, step 64) drip
            # into phase D as PE filler; only mt0/mt1 + V run inline.
            kv_groups = [("k", mt, nb) for mt in range(2)
                         for nb in range(TP)]
            kv_groups += [("v", c, None) for c in range(NKC)]
            for w in range(3):
                grp = kv_groups[w * 8:(w + 1) * 8]
                pss = [accA.tile([128, TOK], F32, tag="acc",
                                 name=f"psA{w}_{i}")
                       for i in range(8)]
                for k in range(KC):
                    for ps, (kind, a, b) in zip(pss, grp):
                        if kind == "k":
                            nc.tensor.matmul(
                                out=ps[:, :],
                                lhsT=wk_sb[k][:, a * 128:(a + 1) * 128],
                                rhs=xcols(k, b * TOK, TOK),
                                start=(k == 0), stop=(k == KC - 1))
                        else:
                            nc.tensor.matmul(
                                out=ps[:, :],
                                lhsT=xcols(k, a * 128, 128),
                                rhs=wv_sb[k][:, :],
                                start=(k == 0), stop=(k == KC - 1))
                for ps, (kind, a, b) in zip(pss, grp):
                    if kind == "k":
                        nc.vector.tensor_copy(out=kTg[b * 4 + a][:, :],
                                              in_=ps[:, :])
                    else:
                        nc.vector.memset(vg[a][:, :], 1.0)
                        for kh in range(NKV):
                            nc.vector.tensor_copy(
                                out=vg[a][:, kh * (HD + 1):kh * (HD + 1) + HD],
                                in_=ps[:, kh * HD:(kh + 1) * HD])

        # =============== phases B+D: pipelined Q chunks + attention ======
        # Q chunk m (q dims m*128..) fills heads 2m, 2m+1. Pair group qg
        # (pairs 4qg..4qg+3) needs exactly Q chunks {4qg, 4qg+2, 4qg+1,
        # 4qg+3}: group 0's chunks are emitted up front; group qg+1's
        # drip into the PE slack of group qg's steps.
        pairs = []
        for g in range(0, NKV, 2):
            for j in range(GRP):
                pairs.append((g * GRP + j, (g + 1) * GRP + j))

        # wop opened early so Wo tiles prefetch during attention
        wqp = es.enter_context(tc.tile_pool(name="wqp", bufs=64))
        wop_box = []

        def wop():
            if not wop_box:
                wop_box.append(es.enter_context(
                    tc.tile_pool(name="wop", bufs=1)))
            return wop_box[0]

        def gen_q_chunk(m):
            """33 thunks: 16 weight DMAs, then 16 matmuls, then a finisher
            (cast + route). The DMA thunks drain ~8 steps ahead of their
            matmuls, so neither the SP queue (no 40us descriptor bursts)
            nor the in-order PE queue ever blocks."""
            ps = accB.tile([128, TOK], F32, tag="accq", name=f"psq{m}")
            ws = [wqp.tile([128, 128], BF, tag="wq", name=f"wq{m}_{k}")
                  for k in range(KC)]
            dmas, mms = [], []
            for k in range(KC):
                def dm(k=k, m=m):
                    nc.sync.dma_start(
                        out=ws[k][:, :],
                        in_=wqT[k * 128:(k + 1) * 128, m * 128:(m + 1) * 128])
                dmas.append(dm)
                def mm(k=k, ps=ps, m=m):
                    nc.tensor.matmul(
                        out=ps[:, :], lhsT=ws[k][:, :], rhs=xq[k][:, :],
                        start=(k == 0), stop=(k == KC - 1))
                mms.append(mm)
            # interleave with a 6-thunk DMA lead so each matmul's weight
            # tile lands ~2 steps before the PE reaches it
            thunks = dmas[:6]
            for k in range(KC):
                if k + 6 < KC:
                    thunks.append(dmas[k + 6])
                thunks.append(mms[k])

            def fin(ps=ps, m=m):
                st = stgB.tile([128, TOK], BF, tag="stg", name=f"stq{m}")
                nc.vector.tensor_copy(out=st[:, :], in_=ps[:, :])
                # route each head to its kv-parity-aligned slot via DMA
                for j in range(2):
                    h = 2 * m + j
                    i, roff = q_slot(h)
                    nc.sync.dma_start(out=qTp[i][roff:roff + 64, :],
                                      in_=st[j * 64:(j + 1) * 64, :])
            thunks.append(fin)
            return thunks

        def gen_k_unit(mt, nb):
            """17 thunks: K-projection group (mt, nb) as phase-D filler
            (wk/xr stay resident; shares the accq PSUM bank ring)."""
            ps = accB.tile([128, TOK], F32, tag="accq", name=f"psk{nb}_{mt}")
            thunks = []
            for k in range(KC):
                def mm(k=k, ps=ps, mt=mt, nb=nb):
                    nc.tensor.matmul(
                        out=ps[:, :],
                        lhsT=wk_sb[k][:, mt * 128:(mt + 1) * 128],
                        rhs=xcols(k, nb * TOK, TOK),
                        start=(k == 0), stop=(k == KC - 1))
                thunks.append(mm)

            def fin(ps=ps, mt=mt, nb=nb):
                nc.vector.tensor_copy(out=kTg[nb * 4 + mt][:, :],
                                      in_=ps[:, :])
            thunks.append(fin)
            return thunks

        # Wo prefetch for phase E's first output-column block (nt=0):
        # dripped into late phase D so phase E starts compute-bound.
        wo_pre = {}

        def gen_wo_prefetch(nt):
            thunks = []

            def last(nt=nt):
                wl = wop().tile([1, 512], BF, tag="wolast", name=f"wl{nt}")
                nc.sync.dma_start(
                    out=wl[:, :],
                    in_=woT[HID:HID + 1, nt * 512:(nt + 1) * 512])
                wo_pre[(nt, "last")] = wl
            thunks.append(last)
            for kc in range(KC):
                def f(kc=kc, nt=nt):
                    w = wop().tile([128, 512], BF, tag="wo", bufs=12, name=f"wo{nt}_{kc}")
                    nc.sync.dma_start(
                        out=w[:, :],
                        in_=woT[kc * 128:(kc + 1) * 128,
                                nt * 512:(nt + 1) * 512])
                    wo_pre[(nt, kc)] = w
                thunks.append(f)
            return thunks

        drip = deque()

        LAG = 2  # steps between scores+exp and the matching PV

        with tc.tile_pool(name="accB", bufs=1, space="PSUM") as accB, \
             tc.tile_pool(name="stgB", bufs=2) as stgB, \
             tc.tile_pool(name="sps", bufs=2, space="PSUM") as sps, \
             tc.tile_pool(name="ops", bufs=2, space="PSUM") as ops, \
             tc.tile_pool(name="bps", bufs=1, space="PSUM") as bps, \
             tc.tile_pool(name="ptp", bufs=6) as ptp, \
             tc.tile_pool(name="nrm", bufs=2) as nrm:

            # chunks 0,2 (pair 0's heads) run inline; everything else
            # queues upfront in consumer-deadline order, K units
            # interleaved between Q chunks so every accq-bank handoff
            # gets >=1 step of DMA-thunk separation. Deadlines verified
            # at 6 pops/step (tightest margin: chunk 10, 6 steps).
            for m in (0, 2):
                for th in gen_q_chunk(m):
                    th()
            sched_bg = [("q", 1), ("q", 3),
                        ("q", 4), ("k", 2, 0), ("q", 6), ("k", 2, 1),
                        ("q", 5), ("k", 2, 2), ("q", 7), ("k", 2, 3),
                        ("q", 8), ("k", 3, 0), ("q", 10), ("k", 3, 1),
                        ("q", 9), ("k", 3, 2), ("q", 11), ("k", 3, 3),
                        ("q", 12), ("q", 14), ("q", 13), ("q", 15)]
            for u in sched_bg:
                if u[0] == "q":
                    drip.extend(gen_q_chunk(u[1]))
                else:
                    drip.extend(gen_k_unit(u[1], u[2]))

            # Normalization for pair p is staged across pair p+1's steps
            # so no PE-side consumer ever waits on the DVE chain: po is
            # copied to SBUF right after the last PV (freeing the PSUM
            # bank a full pair before reuse), the 3.3us reciprocals run
            # two steps before their consumers, and attnT lands by the
            # end of pair p+1 (only pair 15's chain remains in the tail).
            # All per-row work stays on partition 64 (DVE can't shift
            # partitions; DMA can't read PSUM).
            aph_of = {}

            def norm_copy(p, ci):
                # ci == 2 copies head A, ci == 4 copies head B
                j = 0 if ci == 2 else 1
                h = pairs[p][j]
                po = po_of[p][j]
                aph = nrm.tile([HD + 1, TOK], F32, tag="aph", bufs=2,
                               name=f"aph{h}")
                nc.vector.tensor_copy(out=aph[:, :], in_=po[:, :])
                aph_of[(p, j)] = [aph, None, None]
                if ci == 4:
                    del po_of[p]

            def norm_recip(p, j):
                h = pairs[p][j]
                ent = aph_of[(p, j)]
                rcp = nrm.tile([HD + 1, TOK], F32, tag="rcp", name=f"rc{h}")
                nc.vector.reciprocal(out=rcp[HD:HD + 1, :],
                                     in_=ent[0][HD:HD + 1, :])
                ent[1] = rcp

            def norm_cast(p, j):
                h = pairs[p][j]
                ent = aph_of[(p, j)]
                rcpb = nrm.tile([HD + 1, TOK], BF, tag="rcpb",
                                name=f"rb{h}")
                nc.vector.tensor_copy(out=rcpb[HD:HD + 1, :],
                                      in_=ent[1][HD:HD + 1, :])
                ent[1] = rcpb

            def norm_bcast(p, j):
                h = pairs[p][j]
                ent = aph_of[(p, j)]
                pb = bps.tile([64, TOK], F32, tag="pb", name=f"pb{h}")
                nc.tensor.matmul(out=pb[:, :], lhsT=ones64[HD:HD + 1, :],
                                 rhs=ent[1][HD:HD + 1, :],
                                 start=True, stop=True)
                rb = nrm.tile([64, TOK], BF, tag="rbb", name=f"rbb{h}")
                nc.vector.tensor_copy(out=rb[:, :], in_=pb[:, :])
                ent[2] = rb

            def norm_mul(p, j):
                h = pairs[p][j]
                ent = aph_of[(p, j)]
                t, half = h // 2, (h % 2) * 64
                if half == 0:
                    nc.vector.tensor_mul(out=attnT[t][0:64, :],
                                         in0=ent[0][0:HD, :],
                                         in1=ent[2][:, :])
                else:
                    ah = nrm.tile([64, TOK], BF, tag="ah", name=f"ah{h}")
                    nc.vector.tensor_mul(out=ah[:, :], in0=ent[0][0:HD, :],
                                         in1=ent[2][:, :])
                    nc.sync.dma_start(out=attnT[t][64:128, :],
                                      in_=ah[:, :])
                del aph_of[(p, j)]

            # stage schedule for pair p, emitted during pair p+1:
            #   ci4: recip A   ci6: recip B   ci8: cast A+B
            #   ci10: bcast A  ci12: bcast B + mul A   ci14: mul B
            def norm_stage2(p, ci):
                if ci == 4:
                    norm_recip(p, 0)
                elif ci == 6:
                    norm_recip(p, 1)
                elif ci == 8:
                    norm_cast(p, 0)
                    norm_cast(p, 1)
                elif ci == 10:
                    norm_bcast(p, 0)
                elif ci == 12:
                    norm_bcast(p, 1)
                    norm_mul(p, 0)
                elif ci == 14:
                    norm_mul(p, 1)

            def emit_scores(pi, ci):
                hA, hB = pairs[pi]
                kt = (hA // GRP) // 2
                qiA, _ = q_slot(hA)
                qiB, _ = q_slot(hB)
                psA = sps.tile([128, 2 * TOK], F32, tag="ps",
                               name=f"psA{hA}_{ci}")
                psB = sps.tile([128, 2 * TOK], F32, tag="ps",
                               name=f"psB{hB}_{ci}")
                for dc in range(2):
                    c = ci + dc
                    nb, lc = c // 4, c % 4
                    kts = kTg[nb * 4 + kt]
                    nc.tensor.matmul(
                        out=psA[:, dc * TOK:(dc + 1) * TOK],
                        lhsT=kts[0:64, lc * 128:(lc + 1) * 128],
                        rhs=qTp[qiA][0:64, :], start=True, stop=True)
                ptA = ptp.tile([128, 2 * TOK], BF, tag="pt",
                               name=f"ptA{hA}_{ci}")
                nc.scalar.activation(
                    out=ptA[:, :], in_=psA[:, :],
                    func=mybir.ActivationFunctionType.Exp, scale=EXP_SCALE)
                for dc in range(2):
                    c = ci + dc
                    nb, lc = c // 4, c % 4
                    kts = kTg[nb * 4 + kt]
                    nc.tensor.matmul(
                        out=psB[:, dc * TOK:(dc + 1) * TOK],
                        lhsT=kts[64:128, lc * 128:(lc + 1) * 128],
                        rhs=qTp[qiB][64:128, :], start=True, stop=True)
                ptB = ptp.tile([128, 2 * TOK], BF, tag="pt",
                               name=f"ptB{hB}_{ci}")
                nc.scalar.activation(
                    out=ptB[:, :], in_=psB[:, :],
                    func=mybir.ActivationFunctionType.Exp, scale=EXP_SCALE)
                return ptA, ptB

            def emit_pv(pi, ci, ptA, ptB):
                hA, hB = pairs[pi]
                khA, khB = hA // GRP, hB // GRP
                poA, poB = po_of[pi]
                for dc in range(2):
                    c = ci + dc
                    nc.tensor.matmul(
                        out=poA[:, :],
                        lhsT=vg[c][:, khA * (HD + 1):(khA + 1) * (HD + 1)],
                        rhs=ptA[:, dc * TOK:(dc + 1) * TOK],
                        start=(c == 0), stop=(c == NKC - 1))
                    nc.tensor.matmul(
                        out=poB[:, :],
                        lhsT=vg[c][:, khB * (HD + 1):(khB + 1) * (HD + 1)],
                        rhs=ptB[:, dc * TOK:(dc + 1) * TOK],
                        start=(c == 0), stop=(c == NKC - 1))

            sched = [(pi, 2 * c2) for pi in range(len(pairs))
                     for c2 in range(NKC // 2)]
            po_of = {}
            inflight = deque()  # (pi, ci, ptA, ptB) awaiting PV emission

            for s, (pi, ci) in enumerate(sched):
                hA, hB = pairs[pi]
                if ci == 0:
                    poA = ops.tile([HD + 1, TOK], F32, tag="po",
                                   name=f"poA{hA}")
                    poB = ops.tile([HD + 1, TOK], F32, tag="po",
                                   name=f"poB{hB}")
                    po_of[pi] = (poA, poB)
                # PV of step s-LAG goes FIRST: it never waits (its exp
                # finished ~2 steps ago), so it fills the window where
                # scores-A(s) would stall on exp(s-1)-A freeing its PSUM
                # buffer — the PE queue is in-order.
                if len(inflight) >= LAG:
                    emit_pv(*inflight.popleft())
                ptA, ptB = emit_scores(pi, ci)
                inflight.append((pi, ci, ptA, ptB))
                # staged normalization for the previous pair (see above)
                if pi > 0:
                    if ci in (2, 4):
                        norm_copy(pi - 1, ci)
                    if ci >= 4:
                        norm_stage2(pi - 1, ci)
                # drip queued background thunks (DMA thunks are free
                # for the PE; ~3 matmul thunks per step)
                for _ in range(6):
                    if drip:
                        drip.popleft()()

            while inflight:
                emit_pv(*inflight.popleft())
            while drip:
                drip.popleft()()
            # drain the normalization pipeline for the last pair
            for ci in (2, 4):
                norm_copy(15, ci)
            for ci in range(4, 16, 2):
                norm_stage2(15, ci)

        # =============== phase E: output projection + bias ===========
        wop()  # open before yps/ystg so pool closes stay LIFO
        with tc.tile_pool(name="yps", bufs=8, space="PSUM") as yps, \
             tc.tile_pool(name="ystg", bufs=4) as ystg:
            for nt in range(4):        # 4 output column blocks of 512
                if (nt, "last") in wo_pre:
                    wo_last = wo_pre[(nt, "last")]
                else:
                    wo_last = wop().tile([1, 512], BF, tag="wolast",
                                       name=f"wl{nt}")
                    nc.sync.dma_start(
                        out=wo_last[:, :],
                        in_=woT[HID:HID + 1, nt * 512:(nt + 1) * 512])
                pys = [yps.tile([128, 512], F32, tag="py",
                                name=f"py{nt}_{i}") for i in range(4)]
                for kc in range(KC):
                    if (nt, kc) in wo_pre:
                        wo_t = wo_pre[(nt, kc)]
                    else:
                        wo_t = wop().tile([128, 512], BF, tag="wo",
                                          bufs=12, name=f"wo{nt}_{kc}")
                        nc.sync.dma_start(
                            out=wo_t[:, :],
                            in_=woT[kc * 128:(kc + 1) * 128,
                                    nt * 512:(nt + 1) * 512])
                    for mt in range(4):
                        nc.tensor.matmul(
                            out=pys[mt][:, :],
                            lhsT=attnT[kc][:, mt * 128:(mt + 1) * 128],
                            rhs=wo_t[:, :],
                            start=(kc == 0), stop=False)
                for mt in range(4):    # bias via ones row, K=1 matmul
                    nc.tensor.matmul(
                        out=pys[mt][:, :], lhsT=ones128[:, :],
                        rhs=wo_last[:, :], start=False, stop=True)
                    ys = ystg.tile([128, 512], F32, tag="ys",
                                   name=f"ys{nt}_{mt}")
                    nc.vector.tensor_copy(out=ys[:, :], in_=pys[mt][:, :])
                    nc.sync.dma_start(
                        out=out[mt * 128:(mt + 1) * 128,
                                nt * 512:(nt + 1) * 512],
                        in_=ys[:, :])

    nc.finalize()
    return nc


@functools.lru_cache(maxsize=1)
def _graph():
    return build_graph()


def make_in_maps(x, Wq, Wk, Wv, Wo, bo):
    bf16 = ml_dtypes.bfloat16
    x = np.asarray(x, np.float32)
    wqT = np.ascontiguousarray(np.asarray(Wq, np.float32).T).astype(bf16)
    wkT = np.ascontiguousarray(np.asarray(Wk, np.float32).T).astype(bf16)
    wvT = np.ascontiguousarray(np.asarray(Wv, np.float32).T).astype(bf16)
    woT = np.concatenate(
        [np.asarray(Wo, np.float32).T,
         np.asarray(bo, np.float32)[None, :]], axis=0).astype(bf16)
    woT = np.ascontiguousarray(woT)
    in_maps = []
    for c in range(8):
        b, r = c // TP, c % TP
        # token permutation: own query block first, rest after (attention
        # is permutation-invariant over keys)
        perm = np.r_[r * TOK:(r + 1) * TOK, 0:r * TOK, (r + 1) * TOK:S]
        xT_c = np.ascontiguousarray(x[b].T[:, perm]).astype(bf16)
        in_maps.append(
            {"xT": xT_c, "wqT": wqT, "wkT": wkT, "wvT": wvT, "woT": woT})
    return in_maps


def kernel(x, Wq, Wk, Wv, Wo, bo):
    nc = _graph()
    in_maps = make_in_maps(x, Wq, Wk, Wv, Wo, bo)
    res = run_bass_kernel_spmd(nc, in_maps, core_ids=list(range(8)))
    out = np.empty((B, S, HID), np.float32)
    for c in range(8):
        b, r = c // TP, c % TP
        out[b, r * TOK:(r + 1) * TOK, :] = np.asarray(
            res.results[c]["out"], np.float32)
    return out



# revision 5
# speedup vs baseline: 1.0491x; 1.0491x over previous
"""GQA attention (B=2, S=2048, HID=2048, 32 q heads / 8 kv heads, fp32 I/O)
on 8 TRN2 NeuronCores.

Sharding: sequence-parallel with fully local K/V. Core c owns 512 query
tokens of batch c//4 (cores 0-3 = batch 0, cores 4-7 = batch 1), but
computes K^T and V for ALL 2048 tokens of its batch locally — that
(+~40% KV projection FLOPs) is much cheaper than an intra-chip
AllGather, which measures 100-170us and blockades the DMA engines while
it runs. Attention is permutation-invariant over keys, so each core
orders tokens own-block-first (host-side permutation) and the device
program stays rank-independent.

V carries a fused ones-column per kv head so the PV matmul also
produces the softmax row-sums; the output-projection bias is fused as
an extra contraction row. All matmuls run in bf16 with fp32 PSUM
accumulation. Score matmuls (K=64) pair head A (partitions 0-63) and
head B (64-127) on disjoint PE row-groups so the hardware co-executes
them (auto tile_position from base partitions).

v2 schedule: the serial K/V-projection prologue is collapsed to one
8-bank PSUM wave {K mt0 x4, Q0, Q2, V(c0,kv0-3), V(c1,kv0-3)} streamed
chunk-outer against the x DMA arrival, so the first exp fires at
~45us (was ~122us). Everything else — V in 30 finer (chunk, kv-half)
units, K mt1-mt3, Q chunks, Wo prefetch — drips into the ACT-bound
pair loop through a 2-bank ping-pong PSUM ring with deadline-forced,
cost-budgeted pops. The xin pool (x remainder + Wk/Wv) releases at
step 96 to make room for prefetching all 4 Wo column blocks, so the
output projection runs as a pure-PE tail with all 8 PSUM banks.
"""

import functools
from collections import deque
from contextlib import ExitStack

import numpy as np
import ml_dtypes

import concourse.bass as bass
import concourse.mybir as mybir
import concourse.tile as tile
from concourse import bacc
from concourse.bass_utils import run_bass_kernel_spmd

BF = mybir.dt.bfloat16
F32 = mybir.dt.float32

B, S, HID = 2, 2048, 2048
NH, NKV, HD = 32, 8, 64          # q heads, kv heads, head dim
GRP = NH // NKV                  # 4 q heads per kv head
TP = 4                           # cores per batch group
TOK = S // TP                    # 512 local query tokens per core
KC = HID // 128                  # 16 contraction chunks of 128
NKC = S // 128                   # 16 key chunks of 128 (full seq)
VW = NKV * (HD + 1)              # 520: V width incl. ones columns
EXP_SCALE = float(HD) ** -0.5    # 1/8 softmax scale, fused into Exp
LAG = 2                          # steps between scores+exp and its PV


def q_slot(h):
    """qTp tile index and partition base for head h.

    Head h lives at partition base ((h//4)%2)*64 — the same base its kv
    head kh=h//4 occupies inside the kTg tiles, so the scores matmul's
    lhsT and rhs stay partition-aligned (and heads A/B co-execute on
    disjoint PE row groups).
    """
    return ((h // 4) // 2) * 4 + (h % 4), ((h // 4) % 2) * 64


def build_graph():
    nc = bacc.Bacc(None, target_bir_lowering=False, debug=False, num_devices=8)

    xT = nc.declare_dram_parameter("xT", [HID, S], BF, isOutput=False)
    wkT = nc.declare_dram_parameter("wkT", [HID, NKV * HD], BF, isOutput=False)
    wvT = nc.declare_dram_parameter("wvT", [HID, NKV * HD], BF, isOutput=False)
    wqT = nc.declare_dram_parameter("wqT", [HID, HID], BF, isOutput=False)
    woT = nc.declare_dram_parameter("woT", [HID + 1, HID], BF, isOutput=False)
    out = nc.declare_dram_parameter("out", [TOK, HID], F32, isOutput=True)

    with tile.TileContext(nc) as tc, ExitStack() as es:
        pers = es.enter_context(tc.tile_pool(name="pers", bufs=1))

        def T(shape, dtype, *, name):
            return pers.tile(shape, dtype, name=name, tag=name)

        # long-lived SBUF pools first; xin LAST so it can release at
        # step 96 while still top-of-stack among SBUF pools.
        wqp = es.enter_context(tc.tile_pool(name="wqp", bufs=48))
        stgB = es.enter_context(tc.tile_pool(name="stgB", bufs=2))
        nrm = es.enter_context(tc.tile_pool(name="nrm", bufs=2))
        ptp = es.enter_context(tc.tile_pool(name="ptp", bufs=6))

        xq = [T([128, TOK], BF, name=f"xq{k}") for k in range(KC)]

        xin_cm = tc.tile_pool(name="xin", bufs=1)
        xin = xin_cm.__enter__()
        xr = [xin.tile([128, S - TOK], BF, tag=f"xr{k}", name=f"xr{k}")
              for k in range(KC)]
        wk_sb = [xin.tile([128, NKV * HD], BF, tag=f"wk{k}", name=f"wk{k}")
                 for k in range(KC)]
        wv_sb = [xin.tile([128, NKV * HD], BF, tag=f"wv{k}", name=f"wv{k}")
                 for k in range(KC)]

        # weight tiles for wave-0's Q0/Q2 — allocated before the DMA loop
        wq_w0 = {m: [wqp.tile([128, 128], BF, tag="wq", name=f"wq{m}_{k}")
                     for k in range(KC)] for m in (0, 2)}

        # ---- DMA issue order = priority: everything wave 0 needs, by
        # contraction chunk, then wv (first needed ~5us after wave 0).
        for k in range(KC):
            nc.sync.dma_start(out=wk_sb[k][:, :], in_=wkT[k * 128:(k + 1) * 128, :])
            nc.sync.dma_start(out=xq[k][:, :], in_=xT[k * 128:(k + 1) * 128, 0:TOK])
            nc.sync.dma_start(out=xr[k][:, :], in_=xT[k * 128:(k + 1) * 128, TOK:S])
            for m in (0, 2):
                nc.sync.dma_start(
                    out=wq_w0[m][k][:, :],
                    in_=wqT[k * 128:(k + 1) * 128, m * 128:(m + 1) * 128])
        for k in range(KC):
            nc.sync.dma_start(out=wv_sb[k][:, :], in_=wvT[k * 128:(k + 1) * 128, :])

        def xcols(k, lo, n):
            # columns lo..lo+n of the permuted x^T chunk k
            return xq[k][:, lo:lo + n] if lo < TOK \
                else xr[k][:, lo - TOK:lo - TOK + n]

        # row HD (partition 64) is the K=1 lhsT for the row-sum broadcast
        ones64 = T([HD + 1, 64], BF, name="ones64")
        nc.vector.memset(ones64[:, :], 1.0)
        ones128 = T([1, 128], BF, name="ones128")
        nc.vector.memset(ones128[:, :], 1.0)

        # kTg[nb*4+mt]: [128, 512] = K^T rows mt*128.. for key block nb
        # (kv heads 2mt at partitions 0-63, 2mt+1 at 64-127).
        # vg[c]: [128, 520] V_aug rows for key chunk c, ones at col
        # kh*65+64 of each kv head kh.
        kTg = [T([128, TOK], BF, name=f"kTg{i}") for i in range(16)]
        vg = [T([128, VW], BF, name=f"vg{c}") for c in range(NKC)]
        qTp = [T([128, TOK], BF, name=f"qTp{i}") for i in range(NH // 2)]
        attnT = [T([128, TOK], BF, name=f"attnT{t}") for t in range(NH // 2)]

        def q_fin(ps, m):
            st = stgB.tile([128, TOK], BF, tag="stg", name=f"stq{m}")
            nc.vector.tensor_copy(out=st[:, :], in_=ps[:, :])
            for j in range(2):
                h = 2 * m + j
                i, roff = q_slot(h)
                nc.sync.dma_start(out=qTp[i][roff:roff + 64, :],
                                  in_=st[j * 64:(j + 1) * 64, :])

        def v_fin(ps, c, h):
            if h == 0:
                nc.vector.memset(vg[c][:, :], 1.0)
            for kh in range(4 * h, 4 * h + 4):
                nc.vector.tensor_copy(
                    out=vg[c][:, kh * (HD + 1):kh * (HD + 1) + HD],
                    in_=ps[:, (kh - 4 * h) * 64:(kh - 4 * h + 1) * 64])

        # =============== wave 0: the minimal exp-gating work ============
        # 8 PSUM accumulation groups, contraction-chunk OUTER so the PE
        # streams 8 matmuls per arriving x chunk. Completes ~1.7us after
        # the last x chunk lands; first exp fires ~2us later.
        w0 = ([("k", 0, nb) for nb in range(TP)]
              + [("q", 0, None), ("q", 2, None)]
              + [("v", 0, 0), ("v", 1, 0)])
        with tc.tile_pool(name="accA", bufs=8, space="PSUM") as accA:
            pss = [accA.tile([128, TOK], F32, tag="acc", name=f"psA{i}")
                   for i in range(8)]
            for k in range(KC):
                for ps, (kind, a, b) in zip(pss, w0):
                    if kind == "k":
                        nc.tensor.matmul(
                            out=ps[:, :],
                            lhsT=wk_sb[k][:, 0:128],
                            rhs=xcols(k, b * TOK, TOK),
                            start=(k == 0), stop=(k == KC - 1))
                    elif kind == "q":
                        nc.tensor.matmul(
                            out=ps[:, :], lhsT=wq_w0[a][k][:, :],
                            rhs=xq[k][:, :],
                            start=(k == 0), stop=(k == KC - 1))
                    else:
                        nc.tensor.matmul(
                            out=ps[:, 0:256],
                            lhsT=xcols(k, a * 128, 128),
                            rhs=wv_sb[k][:, 0:256],
                            start=(k == 0), stop=(k == KC - 1))
            # evac in exp-gating order: kTg nb0, Q0, Q2, then the rest
            nc.vector.tensor_copy(out=kTg[0][:, :], in_=pss[0][:, :])
            q_fin(pss[4], 0)
            q_fin(pss[5], 2)
            for nb in range(1, TP):
                nc.vector.tensor_copy(out=kTg[nb * 4][:, :],
                                      in_=pss[nb][:, :])
            v_fin(pss[6][:, 0:256], 0, 0)
            v_fin(pss[7][:, 0:256], 1, 0)

        # =============== drip units (fed into the pair loop) ============
        # Each unit: (deadline_step, [(cost_ns, thunk), ...]).
        MM_NS = 216    # 512-col bf16 matmul streaming time
        VMM_NS = 112   # 256-col

        drip = deque()  # (deadline, cost_ns, thunk)
        _units = []     # (deadline, seq, [(cost, thunk), ...])

        def push_unit(deadline, cts):
            _units.append((deadline, len(_units), cts))

        def seal_units():
            # stable-sort by deadline so FIFO head-forcing pops in need
            # order (units are queued grouped by kind, not by deadline)
            for d, _, cts in sorted(_units, key=lambda u: (u[0], u[1])):
                for c, t in cts:
                    drip.append((d, c, t))

        # unit PSUM tiles MUST allocate lazily at pop time: the dps ring
        # orders its WAR handoffs by .tile() call order, which has to
        # match emission order (norm_bcast's pb tiles share the ring).
        def gen_q_chunk(m):
            """16 weight DMAs (6-ahead interleave), 16 matmuls, finisher."""
            box = {}
            ws = [wqp.tile([128, 128], BF, tag="wq", name=f"wq{m}_{k}")
                  for k in range(KC)]
            dmas, mms = [], []
            for k in range(KC):
                def dm(k=k, m=m):
                    nc.sync.dma_start(
                        out=ws[k][:, :],
                        in_=wqT[k * 128:(k + 1) * 128, m * 128:(m + 1) * 128])
                dmas.append((0, dm))
                def mm(k=k, m=m):
                    if k == 0:
                        box["ps"] = dps.tile([128, TOK], F32, tag="dp",
                                             name=f"psq{m}")
                    nc.tensor.matmul(
                        out=box["ps"][:, :], lhsT=ws[k][:, :], rhs=xq[k][:, :],
                        start=(k == 0), stop=(k == KC - 1))
                mms.append((MM_NS, mm))
            thunks = dmas[:6]
            for k in range(KC):
                if k + 6 < KC:
                    thunks.append(dmas[k + 6])
                thunks.append(mms[k])
            thunks.append((0, lambda m=m: q_fin(box["ps"], m)))
            return thunks

        def gen_k_unit(mt, nb):
            box = {}
            thunks = []
            for k in range(KC):
                def mm(k=k, mt=mt, nb=nb):
                    if k == 0:
                        box["ps"] = dps.tile([128, TOK], F32, tag="dp",
                                             name=f"psk{nb}_{mt}")
                    nc.tensor.matmul(
                        out=box["ps"][:, :],
                        lhsT=wk_sb[k][:, mt * 128:(mt + 1) * 128],
                        rhs=xcols(k, nb * TOK, TOK),
                        start=(k == 0), stop=(k == KC - 1))
                thunks.append((MM_NS, mm))
            def fin(mt=mt, nb=nb):
                nc.vector.tensor_copy(out=kTg[nb * 4 + mt][:, :],
                                      in_=box["ps"][:, :])
            thunks.append((0, fin))
            return thunks

        def gen_v_unit(c, h):
            box = {}
            thunks = []
            for k in range(KC):
                def mm(k=k, c=c, h=h):
                    if k == 0:
                        box["ps"] = dps.tile([128, TOK], F32, tag="dp",
                                             name=f"psv{c}_{h}")
                    nc.tensor.matmul(
                        out=box["ps"][:, 0:256],
                        lhsT=xcols(k, c * 128, 128),
                        rhs=wv_sb[k][:, h * 256:(h + 1) * 256],
                        start=(k == 0), stop=(k == KC - 1))
                thunks.append((VMM_NS, mm))
            thunks.append(
                (0, lambda c=c, h=h: v_fin(box["ps"][:, 0:256], c, h)))
            return thunks

        # Wo prefetch (DMA-only): queued when xin releases at step 96.
        wo_pre = {}
        wo2_box = []

        def gen_wo_prefetch(nt):
            thunks = []
            def last(nt=nt):
                wl = wo2_box[0].tile([1, 512], BF, tag="wolast",
                                     bufs=4, name=f"wl{nt}")
                nc.sync.dma_start(
                    out=wl[:, :], in_=woT[HID:HID + 1, nt * 512:(nt + 1) * 512])
                wo_pre[(nt, "last")] = wl
            thunks.append((0, last))
            for kc in range(KC):
                def f(kc=kc, nt=nt):
                    w = wo2_box[0].tile([128, 512], BF, tag="wo", bufs=64,
                                        name=f"wo{nt}_{kc}")
                    nc.sync.dma_start(
                        out=w[:, :],
                        in_=woT[kc * 128:(kc + 1) * 128,
                                nt * 512:(nt + 1) * 512])
                    wo_pre[(nt, kc)] = w
                thunks.append((0, f))
            return thunks

        # =============== the pair loop ==================================
        pairs = []
        for g in range(0, NKV, 2):
            for j in range(GRP):
                pairs.append((g * GRP + j, (g + 1) * GRP + j))

        with tc.tile_pool(name="sps", bufs=2, space="PSUM") as sps, \
             tc.tile_pool(name="ops", bufs=2, space="PSUM") as ops, \
             tc.tile_pool(name="dps", bufs=2, space="PSUM") as dps:

            # drip queue in deadline order
            for c in range(2, NKC):                      # V kv0-3 rest
                push_unit(max(0, c // 2 - 1), gen_v_unit(c, 0))
            push_unit(13, gen_q_chunk(1))
            push_unit(13, gen_q_chunk(3))
            for nb in range(TP):                         # K mt1
                push_unit(28 + 2 * nb, gen_k_unit(1, nb))
            push_unit(29, gen_q_chunk(4))
            push_unit(29, gen_q_chunk(6))
            push_unit(44, gen_q_chunk(5))
            push_unit(44, gen_q_chunk(7))
            for c in range(NKC):                         # V kv4-7
                push_unit(58 + c // 2, gen_v_unit(c, 1))
            for nb in range(TP):                         # K mt2
                push_unit(60 + 2 * nb, gen_k_unit(2, nb))
            push_unit(61, gen_q_chunk(8))
            push_unit(61, gen_q_chunk(10))
            push_unit(76, gen_q_chunk(9))
            push_unit(76, gen_q_chunk(11))
            for nb in range(TP):                         # K mt3
                push_unit(92 + 2 * nb, gen_k_unit(3, nb))
            push_unit(93, gen_q_chunk(12))
            push_unit(93, gen_q_chunk(14))
            push_unit(108, gen_q_chunk(13))
            push_unit(108, gen_q_chunk(15))
            seal_units()

            # Normalization for pair p staged across pair p+1's steps
            # (PSUM->SBUF copy, reciprocal, cast, PE ones-broadcast via
            # the dps ring, multiply) so the 3.3us DVE reciprocal never
            # blocks a PE-side consumer.
            aph_of = {}

            def norm_copy(p, ci):
                j = 0 if ci == 2 else 1
                h = pairs[p][j]
                po = po_of[p][j]
                aph = nrm.tile([HD + 1, TOK], F32, tag="aph", bufs=2,
                               name=f"aph{h}")
                nc.vector.tensor_copy(out=aph[:, :], in_=po[:, :])
                aph_of[(p, j)] = [aph, None, None]
                if ci == 4:
                    del po_of[p]

            def norm_recip(p, j):
                h = pairs[p][j]
                ent = aph_of[(p, j)]
                rcp = nrm.tile([HD + 1, TOK], F32, tag="rcp", name=f"rc{h}")
                nc.vector.reciprocal(out=rcp[HD:HD + 1, :],
                                     in_=ent[0][HD:HD + 1, :])
                ent[1] = rcp

            def norm_cast(p, j):
                h = pairs[p][j]
                ent = aph_of[(p, j)]
                rcpb = nrm.tile([HD + 1, TOK], BF, tag="rcpb", name=f"rb{h}")
                nc.vector.tensor_copy(out=rcpb[HD:HD + 1, :],
                                      in_=ent[1][HD:HD + 1, :])
                ent[1] = rcpb

            def norm_bcast(p, j):
                h = pairs[p][j]
                ent = aph_of[(p, j)]
                pb = dps.tile([128, TOK], F32, tag="dp", name=f"pb{h}")
                nc.tensor.matmul(out=pb[0:64, :], lhsT=ones64[HD:HD + 1, :],
                                 rhs=ent[1][HD:HD + 1, :],
                                 start=True, stop=True)
                rb = nrm.tile([64, TOK], BF, tag="rbb", name=f"rbb{h}")
                nc.vector.tensor_copy(out=rb[:, :], in_=pb[0:64, :])
                ent[2] = rb

            def norm_mul(p, j):
                h = pairs[p][j]
                ent = aph_of[(p, j)]
                t, half = h // 2, (h % 2) * 64
                if half == 0:
                    nc.vector.tensor_mul(out=attnT[t][0:64, :],
                                         in0=ent[0][0:HD, :],
                                         in1=ent[2][:, :])
                else:
                    ah = nrm.tile([64, TOK], BF, tag="ah", name=f"ah{h}")
                    nc.vector.tensor_mul(out=ah[:, :], in0=ent[0][0:HD, :],
                                         in1=ent[2][:, :])
                    nc.sync.dma_start(out=attnT[t][64:128, :], in_=ah[:, :])
                del aph_of[(p, j)]

            def norm_stage2(p, ci):
                if ci == 4:
                    norm_recip(p, 0)
                elif ci == 6:
                    norm_recip(p, 1)
                elif ci == 8:
                    norm_cast(p, 0)
                    norm_cast(p, 1)
                elif ci == 10:
                    norm_bcast(p, 0)
                elif ci == 12:
                    norm_bcast(p, 1)
                    norm_mul(p, 0)
                elif ci == 14:
                    norm_mul(p, 1)

            def emit_scores(pi, ci):
                hA, hB = pairs[pi]
                kt = (hA // GRP) // 2
                qiA, _ = q_slot(hA)
                qiB, _ = q_slot(hB)
                psA = sps.tile([128, 2 * TOK], F32, tag="ps",
                               name=f"psA{hA}_{ci}")
                psB = sps.tile([128, 2 * TOK], F32, tag="ps",
                               name=f"psB{hB}_{ci}")
                for dc in range(2):
                    c = ci + dc
                    nb, lc = c // 4, c % 4
                    kts = kTg[nb * 4 + kt]
                    nc.tensor.matmul(
                        out=psA[:, dc * TOK:(dc + 1) * TOK],
                        lhsT=kts[0:64, lc * 128:(lc + 1) * 128],
                        rhs=qTp[qiA][0:64, :], start=True, stop=True)
                ptA = ptp.tile([128, 2 * TOK], BF, tag="pt",
                               name=f"ptA{hA}_{ci}")
                nc.scalar.activation(
                    out=ptA[:, :], in_=psA[:, :],
                    func=mybir.ActivationFunctionType.Exp, scale=EXP_SCALE)
                for dc in range(2):
                    c = ci + dc
                    nb, lc = c // 4, c % 4
                    kts = kTg[nb * 4 + kt]
                    nc.tensor.matmul(
                        out=psB[:, dc * TOK:(dc + 1) * TOK],
                        lhsT=kts[64:128, lc * 128:(lc + 1) * 128],
                        rhs=qTp[qiB][64:128, :], start=True, stop=True)
                ptB = ptp.tile([128, 2 * TOK], BF, tag="pt",
                               name=f"ptB{hB}_{ci}")
                nc.scalar.activation(
                    out=ptB[:, :], in_=psB[:, :],
                    func=mybir.ActivationFunctionType.Exp, scale=EXP_SCALE)
                return ptA, ptB

            def emit_pv(pi, ci, ptA, ptB):
                hA, hB = pairs[pi]
                khA, khB = hA // GRP, hB // GRP
                poA, poB = po_of[pi]
                for dc in range(2):
                    c = ci + dc
                    nc.tensor.matmul(
                        out=poA[:, :],
                        lhsT=vg[c][:, khA * (HD + 1):(khA + 1) * (HD + 1)],
                        rhs=ptA[:, dc * TOK:(dc + 1) * TOK],
                        start=(c == 0), stop=(c == NKC - 1))
                    nc.tensor.matmul(
                        out=poB[:, :],
                        lhsT=vg[c][:, khB * (HD + 1):(khB + 1) * (HD + 1)],
                        rhs=ptB[:, dc * TOK:(dc + 1) * TOK],
                        start=(c == 0), stop=(c == NKC - 1))

            sched = [(pi, 2 * c2) for pi in range(len(pairs))
                     for c2 in range(NKC // 2)]
            po_of = {}
            inflight = deque()

            for s, (pi, ci) in enumerate(sched):
                hA, hB = pairs[pi]
                if ci == 0:
                    poA = ops.tile([HD + 1, TOK], F32, tag="po",
                                   name=f"poA{hA}")
                    poB = ops.tile([HD + 1, TOK], F32, tag="po",
                                   name=f"poB{hB}")
                    po_of[pi] = (poA, poB)
                # PV of step s-LAG first: it never waits, so it fills the
                # window where scores-A(s) stalls on exp(s-1) freeing the
                # score-PSUM ring (the PE queue is in-order).
                if len(inflight) >= LAG:
                    emit_pv(*inflight.popleft())
                ptA, ptB = emit_scores(pi, ci)
                inflight.append((pi, ci, ptA, ptB))
                if pi > 0:
                    if ci in (2, 4):
                        norm_copy(pi - 1, ci)
                    if ci >= 4:
                        norm_stage2(pi - 1, ci)
                # deadline-forced + budget-capped drip pops
                spent = 0
                while drip and (drip[0][0] <= s + 2 or spent < 900):
                    _, cost, th = drip.popleft()
                    th()
                    spent += cost
                # release xin at step 96 (xr/wk/wv dead) and queue the
                # full Wo prefetch into the freed SBUF.
                if s == 96:
                    xin_cm.__exit__(None, None, None)
                    wo2_box.append(es.enter_context(
                        tc.tile_pool(name="wo2", bufs=1)))
                    for nt in range(4):
                        for i, (cst, th) in enumerate(gen_wo_prefetch(nt)):
                            drip.append((98 + nt * 6 + i // 3, cst, th))

            while inflight:
                emit_pv(*inflight.popleft())
            while drip:
                drip.popleft()[2]()
            for ci in (2, 4):
                norm_copy(15, ci)
            for ci in range(4, 16, 2):
                norm_stage2(15, ci)

        # =============== phase E: output projection + bias ===========
        with tc.tile_pool(name="yps", bufs=8, space="PSUM") as yps, \
             tc.tile_pool(name="ystg", bufs=4) as ystg:
            for nt in range(4):        # 4 output column blocks of 512
                wo_last = wo_pre[(nt, "last")]
                pys = [yps.tile([128, 512], F32, tag="py",
                                name=f"py{nt}_{i}") for i in range(4)]
                for kc in range(KC):
                    wo_t = wo_pre[(nt, kc)]
                    for mt in range(4):
                        nc.tensor.matmul(
                            out=pys[mt][:, :],
                            lhsT=attnT[kc][:, mt * 128:(mt + 1) * 128],
                            rhs=wo_t[:, :],
                            start=(kc == 0), stop=False)
                for mt in range(4):    # bias via ones row, K=1 matmul
                    nc.tensor.matmul(
                        out=pys[mt][:, :], lhsT=ones128[:, :],
                        rhs=wo_last[:, :], start=False, stop=True)
                    ys = ystg.tile([128, 512], F32, tag="ys",
                                   name=f"ys{nt}_{mt}")
                    nc.vector.tensor_copy(out=ys[:, :], in_=pys[mt][:, :])
                    nc.sync.dma_start(
                        out=out[mt * 128:(mt + 1) * 128,
                                nt * 512:(nt + 1) * 512],
                        in_=ys[:, :])

    nc.finalize()
    return nc


@functools.lru_cache(maxsize=1)
def _graph():
    return build_graph()


def make_in_maps(x, Wq, Wk, Wv, Wo, bo):
    bf16 = ml_dtypes.bfloat16
    x = np.asarray(x, np.float32)
    wqT = np.ascontiguousarray(np.asarray(Wq, np.float32).T).astype(bf16)
    wkT = np.ascontiguousarray(np.asarray(Wk, np.float32).T).astype(bf16)
    wvT = np.ascontiguousarray(np.asarray(Wv, np.float32).T).astype(bf16)
    woT = np.concatenate(
        [np.asarray(Wo, np.float32).T,
         np.asarray(bo, np.float32)[None, :]], axis=0).astype(bf16)
    woT = np.ascontiguousarray(woT)
    in_maps = []
    for c in range(8):
        b, r = c // TP, c % TP
        # token permutation: own query block first, rest after (attention
        # is permutation-invariant over keys)
        perm = np.r_[r * TOK:(r + 1) * TOK, 0:r * TOK, (r + 1) * TOK:S]
        xT_c = np.ascontiguousarray(x[b].T[:, perm]).astype(bf16)
        in_maps.append(
            {"xT": xT_c, "wqT": wqT, "wkT": wkT, "wvT": wvT, "woT": woT})
    return in_maps


def kernel(x, Wq, Wk, Wv, Wo, bo):
    nc = _graph()
    in_maps = make_in_maps(x, Wq, Wk, Wv, Wo, bo)
    res = run_bass_kernel_spmd(nc, in_maps, core_ids=list(range(8)))
    out = np.empty((B, S, HID), np.float32)
    for c in range(8):
        b, r = c // TP, c % TP
        out[b, r * TOK:(r + 1) * TOK, :] = np.asarray(
            res.results[c]["out"], np.float32)
    return out
